# revision 19
# baseline (speedup 1.0000x reference)
"""Trainium2 Bass kernel for nn_MultiHeadAttention_2250562863251.

Key algebraic insight: the reference einsum 'mbhi,nbhj->mnbh' contracts i and j
independently, so scores[m,n,b,h] = (sum_i q[m,b,h,i]) * (sum_j k[n,b,h,j]) --
a rank-1 outer product of per-head row-sums. Full Q/K projections are never
needed; only queries @ (per-head-summed Wq) [E,16], computed on host (tiny).

Sharding: 8 cores = 2 (batch) x 4 (head-groups of 4 heads). SPMD program via
run_bass_kernel_spmd; host shards inputs / gathers + reduces outputs.

v3 architecture (v1: DVE+ACT bound at ~197us; v2 showed DVE stt has no 2x
uop -- 1x at any dtype -- so score building must leave the DVE entirely):
  - scores built BY THE PE as K=2 rank-2 matmuls into PSUM:
    sc[n,m] = c_n*qs_m + 1*beta_m with lhsT=[c;1] (fp16, per head+chunk) and
    rhs=[qs;beta] (fp16, per head). Kills the DVE stt AND all 6MB of
    broadcast tiles (qs/beta/beta+tri), fixing the DMA-bound startup.
  - padding mask folded into V (host zeroes masked valuesT columns; tiny
    ones-mask DMA zeroes denominator rows) so exp needs no bias.
  - exp fused per (n-chunk, head-pair) group [128,1024] straight from PSUM.
  - causal triangle: post-exp DVE tensor_mul with a constant [128,128] 0/1
    mask on the single triangular 128-col block of each diagonal chunk.
  - 2-head passes keep PSUM inside 8 banks: sc groups (2 banks x 2 bufs) +
    2 pool banks + 2 out-proj banks.
  - softmax divide from SBUF fp16 at 2x after an ACT evacuation copy whose
    denominator row rides along; reciprocal via [128,8] partition-spread.
  - epilogues software-pipelined one pass behind so DMA round-trips never
    stall an engine stream.
"""
import sys

for _p in ("/opt/trn_rl_repo", "/root/.axon_site/_ro/trn_rl_repo"):
    if _p not in sys.path:
        sys.path.append(_p)

import numpy as np
import ml_dtypes

import concourse.bass as bass
import concourse.mybir as mybir
import concourse.tile as tile
from concourse import bacc
from concourse.bass_utils import run_bass_kernel_spmd

# Problem shapes (hardcoded per contract)
M = 2048   # query positions
N = 2048   # key positions
B = 2
E = 1024
H = 16
DH = 64        # head dim
HL = 4         # heads per core
KL = HL * DH   # 256 local pooled dims
NEG = -1000.0
P = 128
NK = N // P    # 16 n-chunks
T = 4          # m-tiles of 512
MT = 512
NCORES = 8

f32 = mybir.dt.float32
f16 = mybir.dt.float16
bf16 = mybir.dt.bfloat16

_CACHE = {}


def _build_program():
    if "nc" in _CACHE:
        return _CACHE["nc"]
    nc = bacc.Bacc("TRN2", target_bir_lowering=False, debug=False,
                   num_devices=NCORES)

    vt_d = nc.declare_dram_parameter("vt", [P, 4, (E // P) * MT], f16, isOutput=False)
    wvlt_d = nc.declare_dram_parameter("wvlt", [P, (E // P) * KL], f16, isOutput=False)
    wolt_d = nc.declare_dram_parameter("wolt", [P, (KL // P) * E], f16, isOutput=False)
    ck_d = nc.declare_dram_parameter("ck", [3, HL * NK * P], f16, isOutput=False)
    qb_d = nc.declare_dram_parameter("qb", [3, HL * M], f16, isOutput=False)
    ones_d = nc.declare_dram_parameter("onesm", [P, NK * HL], bf16, isOutput=False)
    tri_d = nc.declare_dram_parameter("tri01", [P, P], f32, isOutput=False)
    # blocked output: [ob, t, 128, 512] -> host reassembles to [E, M]
    outp_d = nc.declare_dram_parameter("outp", [E // P, T, P, MT], f32,
                                       isOutput=True)

    with tile.TileContext(nc) as tc:
        with (
            tc.tile_pool(name="const", bufs=1) as const,
            tc.tile_pool(name="vstream", bufs=2) as vstream,
            tc.tile_pool(name="etpool", bufs=3) as etpool,
            tc.tile_pool(name="rspool", bufs=1) as rspool,
            tc.tile_pool(name="ptn", bufs=2) as ptn,
            tc.tile_pool(name="small", bufs=2) as small,
            tc.tile_pool(name="opool", bufs=3) as opool,
            tc.tile_pool(name="dpool", bufs=2, space="DRAM") as dpool,
            tc.tile_pool(name="ps_sc", bufs=2, space="PSUM") as ps_sc,
            tc.tile_pool(name="ps_v", bufs=1, space="PSUM") as ps_v,
            tc.tile_pool(name="ps_pool", bufs=1, space="PSUM") as ps_pool,
        ):
            # ---- resident constants (small ones first so PE starts ASAP) ----
            wvlt_sb = const.tile([P, E // P, KL], f16)
            nc.sync.dma_start(wvlt_sb[:], wvlt_d.rearrange("p (ek d) -> p ek d", ek=E // P))
            # ck/qb duplicated at partitions 0-2 and 32-34 so the two heads
            # of a pass run their K=3 score matmuls in concurrent row groups
            ck_sb = const.tile([35, HL, NK * P], f16)
            nc.sync.dma_start(ck_sb[0:3], ck_d.rearrange("p (h x) -> p h x", h=HL))
            nc.sync.dma_start(ck_sb[32:35], ck_d.rearrange("p (h x) -> p h x", h=HL))
            qb_sb = const.tile([35, HL, M], f16)
            nc.sync.dma_start(qb_sb[0:3], qb_d.rearrange("p (h m) -> p h m", h=HL))
            nc.sync.dma_start(qb_sb[32:35], qb_d.rearrange("p (h m) -> p h m", h=HL))
            tri_sb = const.tile([P, P], f32)
            nc.sync.dma_start(tri_sb[:], tri_d[:, :])

            # v_sb[:, k, h*65 : h*65+64] = v for head h, chunk k; col 64 =
            # 1.0 where key is live, 0.0 where padded (denominator mask).
            v_sb = const.tile([P, NK, HL * (DH + 1)], bf16)
            nc.sync.dma_start(
                v_sb.rearrange("p k (h x) -> p k h x", x=DH + 1)[:, :, :, DH:DH + 1],
                ones_d.rearrange("p (k h x) -> p k h x", k=NK, x=1))

            wolt_sb = const.tile([P, KL // P, E], f16)

            def emit_wolt_dma():
                nc.sync.dma_start(
                    wolt_sb[:], wolt_d.rearrange("p (kb o) -> p kb o", kb=KL // P))

            vt_sb_l = [None] * 4

            def emit_vt_dma(q):
                vt_sb_l[q] = vstream.tile([P, E // P, MT], f16, tag="vt",
                                          name=f"vt{q}")
                nc.sync.dma_start(
                    vt_sb_l[q][:], vt_d[:, q].rearrange("p (ek n) -> p ek n", ek=E // P))

            def emit_vproj(k):
                # one n-chunk of the v projection (interleaved into pass 0)
                q, nk_r = k // 4, k % 4
                vpsf = ps_v.tile([P, MT], f32, tag="ops", name=f"vps{k}")
                vps = vpsf[:, 0:KL]
                for ek in range(E // P):
                    nc.tensor.matmul(
                        vps[:],
                        vt_sb_l[q][:, ek, nk_r * P:(nk_r + 1) * P],
                        wvlt_sb[:, ek, :],
                        start=(ek == 0),
                        stop=(ek == E // P - 1),
                    )
                nc.vector.tensor_copy(
                    out=v_sb[:, k].rearrange("p (h x) -> p h x", x=DH + 1)[:, :, 0:DH],
                    in_=vps.rearrange("p (h x) -> p h x", x=DH),
                )

            # ---- stage 2: 8 passes pi = 2t + hp, heads {2hp, 2hp+1} ----
            # Chunk loop is software-pipelined one step: score matmuls for
            # chunk k-1 are emitted before pool matmuls for chunk k, so the
            # PE works under the exp of chunk k instead of stalling on it.
            pool_sb_l = [None] * 16          # evacuated pools by (t, h)
            pdiv_l = [None] * 16             # divided pools by (t, h)
            rdall_l = [None] * 8             # denom rows by pass
            rsg_l = [None] * 8
            rd2_l = [None] * 8
            ptn2_l = [None] * T
            NPASS = 2 * T

            def emit_pass(pi, fillers):
                t, hp = pi // 2, pi % 2
                heads = (2 * hp, 2 * hp + 1)
                ts = t * MT
                pools = {}
                for j, h in enumerate(heads):
                    pools[h] = ps_pool.tile([DH + 1, MT], f32,
                                            tag=f"pool{(2 * pi + j) % 3}",
                                            name=f"pool_{pi}_{h}")
                ks = list(range(NK - 1, 4 * t - 1, -1))

                def emit_sc(k):
                    pos = k - 4 * t
                    W = MT if pos >= 4 else (pos + 1) * P
                    hoff = MT   # h1 slice offset (own PSUM bank)
                    sc = ps_sc.tile([P, 2 * MT], f32, tag="sc", name=f"sc{pi}_{k}")
                    for j, h in enumerate(heads):
                        bp = 32 * j
                        nc.tensor.matmul(
                            sc[:, j * hoff:j * hoff + W],
                            ck_sb[bp:bp + 3, h, k * P:(k + 1) * P],
                            qb_sb[bp:bp + 3, h, ts:ts + W],
                            start=True, stop=True,
                            tile_position=(bp, 0),
                        )
                    if pos < 4:
                        # pre-exp causal mask: -30000 on the upper triangle
                        # of the last 128-col block (sc bounded by ~2500, so
                        # masked entries exp to exactly 0)
                        lw = pos * P
                        for j in range(2):
                            nc.vector.tensor_add(
                                out=sc[:, j * hoff + lw:j * hoff + lw + P],
                                in0=sc[:, j * hoff + lw:j * hoff + lw + P],
                                in1=tri_sb[:],
                            )
                    et = etpool.tile([P, 2 * MT], bf16, tag="et", name=f"et{pi}_{k}")
                    # one fused exp over both heads' valid ranges; any stale
                    # middle is bounded (post-mask sc <= 1) so its exp is
                    # finite and never read by the pool matmuls
                    nc.scalar.activation(et[:, 0:hoff + W], sc[:, 0:hoff + W],
                                         mybir.ActivationFunctionType.Exp)
                    return (et, hoff)

                def emit_pool(k, eth):
                    et, hoff = eth
                    pos = k - 4 * t
                    W = MT if pos >= 4 else (pos + 1) * P
                    for j, h in enumerate(heads):
                        nc.tensor.matmul(
                            pools[h][:, 0:W],
                            v_sb[:, k, h * (DH + 1):(h + 1) * (DH + 1)],
                            et[:, j * hoff:j * hoff + W],
                            start=(k == NK - 1 or (t == 3 and pos == 3)),
                            stop=(pos == 0),
                        )

                if pi == 0:
                    emit_vt_dma(3)
                    emit_vproj(15)
                    emit_vproj(14)
                    emit_vproj(13)
                et_hist = [(ks[0], emit_sc(ks[0]))]
                for i, k in enumerate(ks[1:], 1):
                    if pi == 0:
                        # stream v-proj a few chunks ahead of its first use
                        vk = k - 2
                        if vk >= 0:
                            if vk % 4 == 3:
                                emit_vt_dma(vk // 4)
                            emit_vproj(vk)
                    et_hist.append((k, emit_sc(k)))
                    if len(et_hist) > 2:
                        k0, et0 = et_hist.pop(0)
                        emit_pool(k0, et0)
                    if fillers:
                        fillers.pop(0)()
                for k0, et0 in et_hist:
                    emit_pool(k0, et0)
                while fillers:
                    fillers.pop(0)()

                # evacuate pools (ACT, fp16; denominator row rides along)
                rdall_l[pi] = dpool.tile([2, MT], f16, tag=f"rd{pi % 2}",
                                         name=f"rdall{pi}")
                for j, h in enumerate(heads):
                    pool_sb = rspool.tile([DH + 1, MT], f16,
                                          tag=f"pool_sb{(4 * t + h) % 8}",
                                          name=f"pool_sb_{pi}_{h}")
                    nc.vector.tensor_copy(out=pool_sb[:], in_=pools[h][:])
                    pool_sb_l[4 * t + h] = pool_sb
                    nc.sync.dma_start(rdall_l[pi][j:j + 1, :], pool_sb[DH:DH + 1, :])
                rsg = small.tile([P, 2 * MT // P], f16, tag=f"rsg{pi % 2}",
                                 name=f"rsg{pi}")
                nc.sync.dma_start(
                    rsg[:], rdall_l[pi].rearrange("a (b x) -> (a b) x", x=2 * MT // P))
                rsg_l[pi] = rsg

            def divide_fillers(pi):
                # reciprocal + divides for pass pi, as fillers one pass later
                t, hp = pi // 2, pi % 2
                heads = (2 * hp, 2 * hp + 1)

                def f_recip():
                    rsgr = small.tile([P, 2 * MT // P], f16, tag=f"rsgr{pi % 2}",
                                      name=f"rsgr{pi}")
                    with nc.allow_low_precision(reason="per-(m,h) softmax scale"):
                        nc.vector.reciprocal(out=rsgr[:], in_=rsg_l[pi][:])
                    rdall2 = dpool.tile([2, MT], f16, tag=f"rd2{pi % 2}",
                                        name=f"rdall2{pi}")
                    nc.sync.dma_start(
                        rdall2.rearrange("a (b x) -> (a b) x", x=2 * MT // P),
                        rsgr[:])
                    rd2_l[pi] = rdall2

                def f_div(j, h):
                    rsb = small.tile([DH, MT], f16, tag=f"rsb{h % 2}",
                                     name=f"rsb{pi}_{h}")
                    nc.sync.dma_start(
                        rsb[:], rd2_l[pi][j][None, :].to_broadcast([DH, MT]))
                    pdiv = ptn.tile([DH, MT], f16, tag=f"pdiv{h}",
                                    name=f"pdiv{pi}_{h}")
                    nc.vector.tensor_mul(
                        out=pdiv[:],
                        in0=pool_sb_l[4 * t + h][0:DH, :],
                        in1=rsb[:],
                    )
                    pdiv_l[4 * t + h] = pdiv

                return [f_recip] + [
                    (lambda j=j, h=h: f_div(j, h)) for j, h in enumerate(heads)]

            def outproj_fillers(t):
                # head pair-merge + 8 out-proj column blocks, as fillers
                def f_merge():
                    pTn2 = ptn.tile([P, KL // P, MT], f16, tag="ptn2",
                                    name=f"ptn2_{t}")
                    for kb in range(KL // P):
                        nc.sync.dma_start(pTn2[0:DH, kb], pdiv_l[4 * t + 2 * kb][:])
                        nc.sync.dma_start(pTn2[DH:P, kb],
                                          pdiv_l[4 * t + 2 * kb + 1][:])
                    ptn2_l[t] = pTn2

                def f_ob(ob):
                    ops = ps_v.tile([P, MT], f32, tag="ops", name=f"ops{t}_{ob}")
                    for kb in range(KL // P):
                        nc.tensor.matmul(
                            ops[:],
                            wolt_sb[:, kb, ob * P:(ob + 1) * P],
                            ptn2_l[t][:, kb, :],
                            start=(kb == 0),
                            stop=(kb == KL // P - 1),
                        )
                    osb = opool.tile([P, MT], f32, tag="osb", name=f"osb{t}_{ob}")
                    nc.vector.tensor_copy(out=osb[:], in_=ops[:])
                    nc.sync.dma_start(outp_d[ob, t], osb[:])

                return [f_merge] + [(lambda ob=ob: f_ob(ob))
                                    for ob in range(E // P)]

            pending_op = []
            for pi in range(NPASS):
                fl = []
                if pi == 1:
                    fl.append(emit_wolt_dma)
                if pi >= 1:
                    fl.extend(divide_fillers(pi - 1))
                if pi >= 2 and pi % 2 == 0:
                    pending_op = outproj_fillers(pi // 2 - 1)
                    fl.extend(pending_op[:5])
                    pending_op = pending_op[5:]
                else:
                    fl.extend(pending_op)
                    pending_op = []
                emit_pass(pi, fl)
            for f in divide_fillers(NPASS - 1):
                f()
            for f in outproj_fillers(T - 1):
                f()

    nc.compile()
    _CACHE["nc"] = nc
    return nc


def _host_prep(queries, keys, values, Wq, bq, Wk, bk, Wv, bv, Wo, bo, in_mask):
    """Host-side prep. Returns (in_maps, fixup, extras)."""
    qs = np.einsum("mbe,he->mbh", queries, Wq.reshape(H, DH, E).sum(1),
                   dtype=np.float32) + bq.reshape(H, DH).sum(1)
    ks = np.einsum("nbe,he->nbh", keys, Wk.reshape(H, DH, E).sum(1),
                   dtype=np.float32) + bk.reshape(H, DH).sum(1)
    # device multiplies fp16-rounded qs and c; compute beta from the same
    qs16 = qs.astype(np.float16)
    qsf = qs16.astype(np.float32)

    mask3 = in_mask[:, :, None]
    c_full = np.where(mask3, 0.0, ks).astype(np.float32)      # [n, b, H]
    chi = c_full.astype(np.float16)
    clo = (c_full - chi.astype(np.float32)).astype(np.float16)
    cpf = chi.astype(np.float32) + clo.astype(np.float32)     # device-exact c

    cmax = np.where(mask3, -np.inf, cpf)
    cmax = np.maximum.accumulate(cmax[::-1], axis=0)[::-1]    # suffix max, n>=m
    cmin = np.where(mask3, np.inf, cpf)
    cmin = np.minimum.accumulate(cmin[::-1], axis=0)[::-1]
    nonempty = np.maximum.accumulate((~in_mask)[::-1], axis=0)[::-1]  # [n, b]

    with np.errstate(invalid="ignore"):
        A = np.where(qsf >= 0, qsf * cmax, qsf * cmin)        # [m, b, H]
    A = np.where(nonempty[:, :, None], A, -np.inf)
    fixup_rows = np.any(~(A > -70.0), axis=2)                 # [m, b] (nan-safe)
    beta = np.where(np.isfinite(A), -A, 1e4)
    beta = np.where(fixup_rows[:, :, None], -1e4, beta)
    beta = beta.astype(np.float32)

    in_maps = []
    def pmajor(a, p=P):
        """[X*p, Y] -> [p, X*Y]: partition-major packing for 1-run-per-
        partition DMA loads matching 'p (x y) -> p x y' device views."""
        X = a.shape[0] // p
        return np.ascontiguousarray(
            a.reshape(X, p, a.shape[1]).transpose(1, 0, 2).reshape(p, -1))

    def pack_vt(vT):
        # [E, N] -> [P, 4, (E//P)*MT]: quarter-major, then ek-major
        a = vT.reshape(E // P, P, 4, MT)          # [ek, p, q, mt]
        return np.ascontiguousarray(
            a.transpose(1, 2, 0, 3).reshape(P, 4, (E // P) * MT))

    # zero masked key columns of v^T: their pooled contribution must vanish
    vt_by_b = []
    for bi in range(B):
        vT = values[:, bi, :].T.copy()
        vT[:, in_mask[:, bi]] = 0.0
        vt_by_b.append(pack_vt(vT.astype(np.float16)))

    # ones-column mask [P, NK, HL]: 1.0 for live keys, 0.0 for padded
    live = (~in_mask).astype(np.float32)                      # [n, b]
    onesm_by_b = [
        np.ascontiguousarray(np.broadcast_to(
            live[:, bi].reshape(NK, P, 1).transpose(1, 0, 2), (P, NK, HL))
        ).reshape(P, NK * HL).astype(ml_dtypes.bfloat16)
        for bi in range(B)]

    # causal mask add: -30000 where p < j (key idx within chunk < query)
    tri01 = np.where(np.arange(P)[:, None] < np.arange(P)[None, :], -30000.0,
                     0.0).astype(np.float32)

    for c in range(NCORES):
        bi, hg = c // 4, c % 4
        lh = slice(hg * HL, (hg + 1) * HL)
        ds = slice(hg * KL, (hg + 1) * KL)
        # ck [3, HL*N]: rows = c_hi, c_lo (fp16 split of c), ones
        ckh = np.empty((3, HL, N), np.float16)
        ckh[0] = chi[:, bi, lh].T
        ckh[1] = clo[:, bi, lh].T
        ckh[2] = 1.0
        # qb [3, HL*M]: rows = qs, qs, beta
        qbh = np.empty((3, HL, M), np.float16)
        qbh[0] = qs16[:, bi, lh].T
        qbh[1] = qs16[:, bi, lh].T
        qbh[2] = beta[:, bi, lh].T.astype(np.float16)
        in_maps.append({
            "vt": vt_by_b[bi],
            "wvlt": pmajor(Wv[ds, :].T.astype(np.float16)),
            "wolt": pmajor(Wo[:, ds].T.astype(np.float16)),
            "ck": np.ascontiguousarray(ckh.reshape(3, HL * N)),
            "qb": np.ascontiguousarray(qbh.reshape(3, HL * M)),
            "onesm": onesm_by_b[bi],
            "tri01": tri01,
        })
    return in_maps, fixup_rows, (qsf, ks)


def _fixup_row(out, m, bi, qs, ks, values, Wv, bv, Wo, bo, in_mask):
    """Exact numpy recompute of one output row (degenerate / extreme rows)."""
    pot = qs[m, bi, :][None, :] * ks[:, bi, :]                # [n, H]
    pot = np.where(in_mask[:, bi][:, None], NEG, pot)
    causal = np.arange(N) < m                                 # mask n < m
    pot = np.where(causal[:, None], NEG, pot)
    pot = pot - pot.max(axis=0, keepdims=True)
    w = np.exp(pot)
    w = w / w.sum(axis=0, keepdims=True)                      # [n, H]
    v = (values[:, bi, :] @ Wv.T + bv).reshape(N, H, DH)
    pooled = np.einsum("nh,nhd->hd", w, v).reshape(E)
    out[m, bi, :] = pooled @ Wo.T + bo


def kernel(queries, keys, values, Wq, bq, Wk, bk, Wv, bv, Wo, bo, in_mask,
           _trace=False):
    args = (queries, keys, values, Wq, bq, Wk, bk, Wv, bv, Wo, bo)
    args = tuple(np.asarray(a, np.float32) for a in args)
    in_mask = np.asarray(in_mask, bool)
    (queries, keys, values, Wq, bq, Wk, bk, Wv, bv, Wo, bo) = args

    nc = _build_program()
    in_maps, fixup_rows, (qs, ks) = _host_prep(
        queries, keys, values, Wq, bq, Wk, bk, Wv, bv, Wo, bo, in_mask)

    res = run_bass_kernel_spmd(nc, in_maps, list(range(NCORES)), trace=_trace)
    results = res.results

    out = np.zeros((M, B, E), np.float32)
    for c in range(NCORES):
        bi = c // 4
        blk = np.asarray(results[c]["outp"], np.float32)   # [8, 4, 128, 512]
        outT = blk.transpose(0, 2, 1, 3).reshape(E, M)
        out[:, bi, :] += outT.T
    out += (bo + bv @ Wo.T)[None, None, :]

    for m, bi in zip(*np.nonzero(fixup_rows)):
        _fixup_row(out, m, bi, qs, ks, values, Wv, bv, Wo, bo, in_mask)

    if _trace:
        return out, res
    return out
